# revision 42
# baseline (speedup 1.0000x reference)
"""Trainium2 Bass kernel for nn_AttentionBlock (B=8, S=2048, D=512, f32).

Strategy: data-parallel over batch — one batch element per NeuronCore (8 cores,
same NEFF, SPMD). Per core, the full attention block is computed with the
"transposed scores" layout so no on-chip transposes are needed.

Key algebraic reduction: scores = q k^T = (x Wq^T)(x Wk^T)^T = x (Wq^T Wk) x^T,
so the host folds M = Wq^T Wk * 1/sqrt(D) into ONE [D, D] matrix and the device
computes a single projection g instead of separate q and k — this removes the
whole k-projection (64 of 704 matmul units, ~15 us of PE time):

  host prep:  xt = x[b].T               [D, S]   (contiguous)
              wm = (Wq.T @ Wk * 1/sqrt(D))      [D, D]
              wv = Wv.T                  [D, D]
  stage A:    gT[e, s] = sum_d wm[d, e] * xt[d, s]     (PSUM accum over d)
  stage B:    sT[k, q] = sum_e xt[e, k] * gT[e, q]     (scores, transposed;
              p[k, q]  = exp(sT)         xt doubles as the key operand)
              l[q]     = sum_k p[k, q]  (DVE partial sums over kb blocks +
                         GpSimd partition_all_reduce into broadcast layout)
  stage C:    v[s, e]  = sum_d xt[d, s] * wv[d, e]     (natural [S, D] layout)
              outT[e, q] = sum_k v[k, e] * p[k, q]
              outT *= 1/l  (DVE reciprocal_approx_fast on the broadcast sums)
  host post:  out[b] = outT.T

Matmuls run as float32r (fp32 storage, single-pass reduced-precision PE mode —
measured ~227 ns per 128x128x512 steady-state, same rate as bf16, ~3.5e-4
end-to-end error; fp8 was measured at 5.7e-2+ rel err, over the 2e-2 gate,
and bf16 at 5e-3 with no speed gain).
Softmax skips max-subtraction: scaled scores for this problem stay within ±10
(exp <= 2.2e4, safely inside the fp32 envelope), which is mathematically
identical to the max-subtracted softmax.

Emission order is tuned so the PE never waits (measured ~1.2-2us total
PE-array idle in-span): full-array warmup matmuls on zeros ramp the HAM clock
while inputs DMA in (the DVFS ramp responds to PE power, not just busy time —
[1,1] warmups left the first ~30 real matmuls 30-100% slow); the
g-projection's later s-chunks are threaded between the first score blocks so
the PE consumes xt at the rate the DMAs deliver it; the v-projection fills the
gap between scores(qc=0) and PV(qc=0) in rotated order with all its
PSUM->SBUF copies on the scalar engine (copies on the DVE stall PV(0)'s first
PSUM bank behind the in-order DVE queue); the whole denominator chain runs on
GpSimd+DVE, entirely off the PE. xt and outT are pre-tiled chunk-major in
DRAM so every 256KB chunk DMA is one dense burst (strided 2KB lines measurably
throttle the DMA-gated early window). The final output tile is written as two
half-tiles but NOT more: each extra dma_start trigger costs ~0.5us at the
tail drain.

Note on timing variance: the device clock is a per-process lottery (~227ns vs
~272ns per matmul observed for identical NEFFs, all engines scaling together);
comparisons between kernel versions must be made on matmul-duration-normalized
traces or best-of-N runs.
"""

import math

import numpy as np

import concourse.mybir as mybir
import concourse.tile as tile
from concourse import bacc, bass_isa
from concourse.bass_utils import run_bass_kernel_spmd

P = 128          # partitions
S = 2048         # sequence length
DM = 512         # d_model == d_attn == d_value
ND = DM // P     # 4  d-model chunks
NS = S // P      # 16 sequence blocks
QC = 512         # q-chunk width for fused score/PV stages
NQC = S // QC    # 4
NEC = DM // P    # 4  e-chunks of the output
N_WARM_TINY = 16  # [1,1] warmup matmuls issued while the zero warm tiles memset
N_WARM_BIG = 12   # full-array [128x128x512] warmup matmuls on zeros: exercises
                  # the whole PE (power-level DVFS ramp signal, not just
                  # busy-time) while the first 1.25MB of inputs land in SBUF

F32 = mybir.dt.float32
F32R = mybir.dt.float32r
BF16 = mybir.dt.bfloat16

# 'f32r' (default): f32 storage, float32r matmuls.  'bf16': bf16 storage+matmuls.
MODE = "f32r"

_NC_CACHE = {}


def _build(mode):
    # tensors feeding the tensor engine carry the matmul dtype: the BIR
    # verifier requires fp32r matmul operands to be *produced* as float32r
    sb_dt = BF16 if mode == "bf16" else F32R
    nc = bacc.Bacc()

    # xt and outT live in DRAM pre-tiled chunk-major (host does the reshapes):
    # row ((i*NQC + sc)*P + p) of xt_d holds xt[i*P + p, sc*QC : (sc+1)*QC],
    # so every [P, QC] chunk DMA is one dense 256KB block instead of 128
    # 2KB lines strided 8KB apart — measurably better HBM burst efficiency
    # in the DMA-gated early window
    xt_d = nc.dram_tensor("xt", [ND * NQC * P, QC], sb_dt, kind="ExternalInput")
    # wm likewise pre-tiled per (i, j) block: row ((i*ND + j)*P + p) holds
    # wm[i*P + p, j*P:(j+1)*P] — dense 64KB loads, and separate semaphores
    # per j so g(sc0, j1) waits only on the j1 blocks, not all of wm's rest
    wm_d = nc.dram_tensor("wm", [ND * ND * P, P], sb_dt, kind="ExternalInput")
    wv_d = nc.dram_tensor("wv", [DM, DM], sb_dt, kind="ExternalInput")
    outT_d = nc.dram_tensor("outT", [NEC * NQC * P, QC], F32, kind="ExternalOutput")

    mm = nc.tensor.matmul

    # f32r outputs on DVE ops trip the low-precision guard; actual matmul
    # accumulation stays in fp32 PSUM throughout.
    with nc.allow_low_precision(reason="fp32r operand rounding; PSUM accumulation is fp32"), \
         tile.TileContext(nc) as tc:
        with tc.tile_pool(name="consts", bufs=1) as consts:
            # persistent SBUF tensors (distinct tags so nothing shares slots)
            wm_sb = [consts.tile([P, DM], sb_dt, name=f"wm{i}", tag=f"wm{i}") for i in range(ND)]
            wv_sb = [consts.tile([P, DM], sb_dt, name=f"wv{i}", tag=f"wv{i}") for i in range(ND)]
            xt_sb = [consts.tile([P, S], sb_dt, name=f"xt{i}", tag=f"xt{i}") for i in range(ND)]
            qt_sb = [consts.tile([P, S], sb_dt, name=f"qt{j}", tag=f"qt{j}") for j in range(ND)]
            v_sb = [consts.tile([P, DM], sb_dt, name=f"v{b}", tag=f"v{b}") for b in range(NS)]
            # fp32 ones used by the tiny warmup matmuls (fp32r forbids
            # free-dim-1 matmuls)
            warm_src = consts.tile([P, 1], F32, name="warm_src", tag="warm_src")
            nc.vector.memset(warm_src, 1.0)
            # zero [P,P]x[P,QC] operands for the full-array warmup matmuls
            # (memset can't write f32r, so memset f32 then cast)
            warm_lhs_f = consts.tile([P, P], F32, name="warm_lhs_f", tag="warm_lhs_f")
            warm_rhs_f = consts.tile([P, QC], F32, name="warm_rhs_f", tag="warm_rhs_f")
            warm_lhs = consts.tile([P, P], sb_dt, name="warm_lhs", tag="warm_lhs")
            warm_rhs = consts.tile([P, QC], sb_dt, name="warm_rhs", tag="warm_rhs")
            nc.vector.memset(warm_lhs_f, 0.0)
            nc.vector.memset(warm_rhs_f, 0.0)
            nc.vector.tensor_copy(warm_lhs, warm_lhs_f)
            nc.vector.tensor_copy(warm_rhs, warm_rhs_f)
            # preload the ACT Exp table during stage A — otherwise the first
            # exp of the scores stage pays the ~1.3us table load inline
            exp_warm = consts.tile([P, 1], F32, name="exp_warm", tag="exp_warm")
            nc.scalar.activation(out=exp_warm, in_=warm_src,
                                 func=mybir.ActivationFunctionType.Exp)

            # input DMAs in first-use order: the first gT psum group needs
            # only wm's j0 columns + the first xt chunk, so those go first
            def xt_rows(i, sc):
                return slice((i * NQC + sc) * P, (i * NQC + sc + 1) * P)

            # xt-sc0 ahead of wm-rest: xt's first chunk gates the FIRST real
            # matmuls right at warmup end (loading all of wm first was
            # measured to starve them 2.9us AND let the clock decay); the
            # later-needed wm columns ride behind and their residual wait is
            # bridged by pad warmups below
            def wm_rows(i, j):
                return slice((i * ND + j) * P, (i * ND + j + 1) * P)

            for i in range(ND):
                nc.sync.dma_start(out=wm_sb[i][:, 0:P], in_=wm_d[wm_rows(i, 0), :])
            for i in range(ND):
                nc.sync.dma_start(out=xt_sb[i][:, 0:QC], in_=xt_d[xt_rows(i, 0), :])
            for j in range(1, ND):
                for i in range(ND):
                    nc.sync.dma_start(out=wm_sb[i][:, j * P:(j + 1) * P],
                                      in_=wm_d[wm_rows(i, j), :])
            for sc in range(1, NQC):
                for i in range(ND):
                    nc.sync.dma_start(
                        out=xt_sb[i][:, sc * QC:(sc + 1) * QC],
                        in_=xt_d[xt_rows(i, sc), :],
                    )
            for i in range(ND):
                nc.sync.dma_start(out=wv_sb[i], in_=wv_d[i * P:(i + 1) * P, :])

            # ---- stage A: g projection (s-chunk-major: the first groups
            # only need wm's j0 columns + the first xt chunk) ----------------
            # psA takes 4 banks and is released before psO opens; psS is
            # opened while psA is still live so it gets never-used banks and
            # carries NO dependency on psA's release (a pool release waits on
            # ALL of the pool's accessors, which otherwise stalls the first
            # scores matmul behind the last stage-A copy)
            from contextlib import ExitStack as _ExitStack
            with (
                tc.tile_pool(name="psS", bufs=3, space="PSUM") as psS,
            ):
                _psa_stack = _ExitStack()
                psA = _psa_stack.enter_context(tc.tile_pool(name="psA", bufs=4, space="PSUM"))
                # PE warmup: tiny matmuls bridge the ~1us until the zero warm
                # tiles are memset, then full-array matmuls keep the whole PE
                # drawing power while inputs stream in, so the HAM clock is
                # ramped when real matmuls start.  (shares the psA tag/slots)
                warm = psA.tile([1, 1], F32, name="warm", tag="psA")
                for w in range(N_WARM_TINY):
                    mm(warm, warm_src, warm_src, start=True, stop=True)
                for w in range(N_WARM_BIG):
                    ps_w = psA.tile([P, QC], F32, name="ps_w", tag="psA")
                    mm(ps_w, warm_lhs, warm_rhs, start=True, stop=True)

                def g_group(sc, j):
                    # gT[:, sc-chunk] j-block; copies alternate ACT/DVE so
                    # neither engine queues deep behind the scores-stage exps
                    ps = psA.tile([P, QC], F32, name="psA", tag="psA")
                    for i in range(ND):
                        mm(ps, wm_sb[i][:, j * P:(j + 1) * P],
                           xt_sb[i][:, sc * QC:(sc + 1) * QC],
                           start=(i == 0), stop=(i == ND - 1))
                    copy_op = nc.scalar.copy if j % 2 == 0 else nc.vector.tensor_copy
                    copy_op(qt_sb[j][:, sc * QC:(sc + 1) * QC], ps)

                # only gT's first s-chunk is computed up front: scores(qc=0)
                # needs just that, and the remaining g groups are threaded
                # into the qc=0 scores loop below so the PE consumes xt at
                # the rate the input DMAs deliver it instead of stalling.
                # Two pad warmups after the first group bridge the systematic
                # ~1.3us wait for wm's later columns (a DMA delivery lull) —
                # an idle PE there was measured to drop the clock back to
                # mid-pstate
                for j in range(ND):
                    g_group(0, j)
                    if j == 0:
                        # 2 pads measured best; 5 just moved the residual
                        # DMA-floor wait elsewhere
                        for w in range(2):
                            ps_w = psA.tile([P, QC], F32, name="ps_w", tag="psA")
                            mm(ps_w, warm_lhs, warm_rhs, start=True, stop=True)

                # ---- stages B+C: scores -> exp -> denominators -> PV ------
                _pso_stack = _ExitStack()
                with (
                    tc.tile_pool(name="ptp", bufs=1) as ptp,
                    tc.tile_pool(name="work", bufs=2) as work,
                    tc.tile_pool(name="outp", bufs=3) as outp,
                ):
                    psO = None
                    for qc in range(NQC):
                        qs = slice(qc * QC, (qc + 1) * QC)
                        pt = ptp.tile([P, NS, QC], sb_dt, name="pt", tag="pt")
                        # partial k-sums of p, computed on the DVE in four
                        # quarters so the last one lands just after the scores
                        # finish and the combined sum is ready for the GpSimd
                        # partition_all_reduce before the PV norms need 1/l
                        h1 = work.tile([P, QC, 1], F32, name="h1", tag="h1", bufs=1)
                        h2 = work.tile([P, QC, 1], F32, name="h2", tag="h2", bufs=1)
                        NQ4 = NS // 4
                        for kb in range(NS):
                            ps_s = psS.tile([P, QC], F32, name="ps_s", tag="ps_s")
                            for j in range(ND):
                                mm(ps_s, xt_sb[j][:, kb * P:(kb + 1) * P], qt_sb[j][:, qs],
                                   start=(j == 0), stop=(j == ND - 1))
                            nc.scalar.activation(out=pt[:, kb, :], in_=ps_s,
                                                 func=mybir.ActivationFunctionType.Exp)
                            if kb == NQ4 - 1:
                                nc.vector.reduce_sum(
                                    out=h1, in_=pt[:, 0:NQ4, :].rearrange("p b q -> p q b"),
                                    axis=mybir.AxisListType.X)
                            elif kb == 2 * NQ4 - 1:
                                nc.vector.reduce_sum(
                                    out=h2, in_=pt[:, NQ4:2 * NQ4, :].rearrange("p b q -> p q b"),
                                    axis=mybir.AxisListType.X)
                                nc.vector.tensor_add(h1[:, :, 0], h1[:, :, 0], h2[:, :, 0])
                            elif kb == 3 * NQ4 - 1:
                                nc.vector.reduce_sum(
                                    out=h2, in_=pt[:, 2 * NQ4:3 * NQ4, :].rearrange("p b q -> p q b"),
                                    axis=mybir.AxisListType.X)
                                nc.vector.tensor_add(h1[:, :, 0], h1[:, :, 0], h2[:, :, 0])
                            if qc == 0 and kb % NQ4 == NQ4 - 1 and kb != NS - 1:
                                # thread the next gT s-chunk between score
                                # blocks: fills the PE while xt streams in
                                for j in range(ND):
                                    g_group(kb // NQ4 + 1, j)
                                if kb == NS - NQ4 - 1:
                                    # all g groups emitted: free psA's four
                                    # banks and only now open psO on them
                                    _psa_stack.close()
                                    psO = _pso_stack.enter_context(
                                        tc.tile_pool(name="psO", bufs=5, space="PSUM"))

                        if qc == 0:
                            # v-projection, emitted here so it fills the PE while
                            # the qc=0 exps finish (PV(0) depends on all of them).
                            # Emission is rotated so v_sb[0..5] — the tiles the
                            # first PV group consumes during its first ~1.4us —
                            # are produced mid-stream, not last: their copies are
                            # done before PV(0) streams past them, so PV starts
                            # the instant the last v matmul retires
                            for b in [(i + 6) % NS for i in range(NS)]:
                                psv = psO.tile([P, DM], F32, name="psv", tag="ps_o")
                                for i in range(ND):
                                    mm(psv, xt_sb[i][:, b * P:(b + 1) * P], wv_sb[i],
                                       start=(i == 0), stop=(i == ND - 1))
                                # ALL copies on ACT (idle in this window — the
                                # qc=0 exps are long done): putting half on the
                                # DVE was measured to stall PV(0)'s first PSUM
                                # bank ~550ns AND push the Q4-reduce/reciprocal
                                # chain to the wire, because the DVE drains its
                                # queue in order behind the trailing v copies
                                nc.scalar.copy(v_sb[b], psv)

                        nc.vector.reduce_sum(
                            out=h2, in_=pt[:, 3 * NQ4:NS, :].rearrange("p b q -> p q b"),
                            axis=mybir.AxisListType.X)
                        nc.vector.tensor_add(h1[:, :, 0], h1[:, :, 0], h2[:, :, 0])

                        # denominator: l[q] = sum over partitions of h1, done as
                        # a GpSimd partition_all_reduce (idle engine, SBUF-only)
                        # into a broadcast layout, then one fast DVE reciprocal.
                        # No PE work and no 3.3us exact-reciprocal serialization;
                        # ~51-ULP accuracy is far inside the error budget.
                        l_bc = work.tile([P, QC], F32, name="l_bc", tag="l_bc", bufs=1)
                        nc.gpsimd.partition_all_reduce(
                            l_bc, h1[:, :, 0], channels=P,
                            reduce_op=bass_isa.ReduceOp.add)
                        r_bc = work.tile([P, QC], F32, name="r_bc", tag="r_bc")
                        nc.vector.reciprocal_approx_fast(out=r_bc, in_=l_bc)
                        last = qc == NQC - 1
                        # PV: outT[e, q] = sum_k v[k, e] * p[k, q]; each chunk is
                        # normalized on the DVE and DMA'd out as soon as its PSUM
                        # group completes
                        for ec in range(NEC):
                            if last and ec == NEC - 1:
                                # final output tile in two column halves (separate
                                # PSUM banks — a shared bank would serialize on the
                                # first half's norm read): the first half's
                                # norm+DMA overlap the second half's matmuls
                                for h in range(2):
                                    hs = slice(h * (QC // 2), (h + 1) * (QC // 2))
                                    ps_h = psO.tile([P, QC // 2], F32, name="ps_h", tag="ps_o")
                                    for kb in range(NS):
                                        mm(ps_h, v_sb[kb][:, ec * P:(ec + 1) * P],
                                           pt[:, kb, hs], start=(kb == 0), stop=(kb == NS - 1))
                                    out_h = outp.tile([P, QC // 2], F32, name="out_h", tag="out_h")
                                    nc.vector.tensor_mul(out_h, ps_h, r_bc[:, hs])
                                    nc.sync.dma_start(
                                        out=outT_d[(ec * NQC + qc) * P:(ec * NQC + qc + 1) * P, hs],
                                        in_=out_h)
                            else:
                                ps_o = psO.tile([P, QC], F32, name="ps_o", tag="ps_o")
                                for kb in range(NS):
                                    mm(ps_o, v_sb[kb][:, ec * P:(ec + 1) * P], pt[:, kb, :],
                                       start=(kb == 0), stop=(kb == NS - 1))
                                out_sb = outp.tile([P, QC], F32, name="out_sb", tag="out_sb")
                                nc.vector.tensor_mul(out_sb, ps_o, r_bc)
                                nc.sync.dma_start(
                                    out=outT_d[(ec * NQC + qc) * P:(ec * NQC + qc + 1) * P, :],
                                    in_=out_sb)
                    _pso_stack.close()

    nc.compile()
    return nc


def _get_nc(mode):
    if mode not in _NC_CACHE:
        _NC_CACHE[mode] = _build(mode)
    return _NC_CACHE[mode]


def _prep_in_maps(x, Wq, Wk, Wv, mode):
    if mode == "bf16":
        import ml_dtypes

        def cast(a):
            return np.ascontiguousarray(a).astype(ml_dtypes.bfloat16)
    else:
        def cast(a):
            return np.ascontiguousarray(a, dtype=np.float32)

    scale = 1.0 / math.sqrt(DM)
    # fold the whole score bilinear form into one matrix: scores = x (Wq^T Wk s) x^T
    wm_h = cast((np.asarray(Wq, np.float32).T @ np.asarray(Wk, np.float32)) * scale)
    wv_h = cast(np.asarray(Wv, np.float32).T)
    x = np.asarray(x, np.float32)

    def tile_xt(xb):
        # [D, S] -> chunk-major [(i, sc, p), q]: each [P, QC] chunk one dense block
        return cast(xb.T.reshape(ND, P, NQC, QC).transpose(0, 2, 1, 3)
                    .reshape(ND * NQC * P, QC))

    # wm -> per-(i, j) block-major, dense 64KB blocks
    wm_t = np.ascontiguousarray(
        np.asarray(wm_h, dtype=wm_h.dtype).reshape(ND, P, ND, P)
        .transpose(0, 2, 1, 3).reshape(ND * ND * P, P))

    return [
        {"xt": tile_xt(x[b]), "wm": wm_t, "wv": wv_h}
        for b in range(x.shape[0])
    ]


def _untile_out(outT_tiled):
    # chunk-major [(ec, qc, p), q] -> [S, D] (one batch element, transposed back)
    return (outT_tiled.reshape(NEC, NQC, P, QC).transpose(0, 2, 1, 3)
            .reshape(DM, S).T)


def _run(in_maps, mode=None, **kw):
    mode = mode or MODE
    nc = _get_nc(mode)
    return run_bass_kernel_spmd(nc, in_maps, core_ids=list(range(len(in_maps))), **kw)


def kernel(x, Wq, Wk, Wv):
    in_maps = _prep_in_maps(x, Wq, Wk, Wv, MODE)
    res = _run(in_maps)
    out = np.stack([_untile_out(r["outT"]) for r in res.results])
    return np.ascontiguousarray(out, dtype=np.float32)


# revision 45
# speedup vs baseline: 1.0041x; 1.0041x over previous
"""Trainium2 Bass kernel for nn_AttentionBlock (B=8, S=2048, D=512, f32).

Strategy: data-parallel over batch — one batch element per NeuronCore (8 cores,
same NEFF, SPMD). Per core, the full attention block is computed with the
"transposed scores" layout so no on-chip transposes are needed.

Key algebraic reduction: scores = q k^T = (x Wq^T)(x Wk^T)^T = x (Wq^T Wk) x^T,
so the host folds M = Wq^T Wk * 1/sqrt(D) into ONE [D, D] matrix and the device
computes a single projection g instead of separate q and k — this removes the
whole k-projection (64 of 704 matmul units, ~15 us of PE time):

  host prep:  xt = x[b].T               [D, S]   (contiguous)
              wm = (Wq.T @ Wk * 1/sqrt(D))      [D, D]
              wv = Wv.T                  [D, D]
  stage A:    gT[e, s] = sum_d wm[d, e] * xt[d, s]     (PSUM accum over d)
  stage B:    sT[k, q] = sum_e xt[e, k] * gT[e, q]     (scores, transposed;
              p[k, q]  = exp(sT)         xt doubles as the key operand)
              l[q]     = sum_k p[k, q]  (DVE partial sums over kb blocks +
                         GpSimd partition_all_reduce into broadcast layout)
  stage C:    v[s, e]  = sum_d xt[d, s] * wv[d, e]     (natural [S, D] layout)
              outT[e, q] = sum_k v[k, e] * p[k, q]
              outT *= 1/l  (DVE reciprocal_approx_fast on the broadcast sums)
  host post:  out[b] = outT.T

Matmuls run as float32r (fp32 storage, single-pass reduced-precision PE mode —
measured ~227 ns per 128x128x512 steady-state, same rate as bf16, ~3.5e-4
end-to-end error; fp8 was measured at 5.7e-2+ rel err, over the 2e-2 gate,
and bf16 at 5e-3 with no speed gain).
Softmax skips max-subtraction: scaled scores for this problem stay within ±10
(exp <= 2.2e4, safely inside the fp32 envelope), which is mathematically
identical to the max-subtracted softmax.

Emission order is tuned so the PE never waits (measured ~1.2-2us total
PE-array idle in-span): full-array warmup matmuls on zeros ramp the HAM clock
while inputs DMA in (the DVFS ramp responds to PE power, not just busy time —
[1,1] warmups left the first ~30 real matmuls 30-100% slow); the
g-projection's later s-chunks are threaded between the first score blocks so
the PE consumes xt at the rate the DMAs deliver it; the v-projection fills the
gap between scores(qc=0) and PV(qc=0) in rotated order with all its
PSUM->SBUF copies on the scalar engine (copies on the DVE stall PV(0)'s first
PSUM bank behind the in-order DVE queue); the whole denominator chain runs on
GpSimd+DVE, entirely off the PE. xt and outT are pre-tiled chunk-major in
DRAM so every 256KB chunk DMA is one dense burst (strided 2KB lines measurably
throttle the DMA-gated early window). The final output tile is written as two
half-tiles but NOT more: each extra dma_start trigger costs ~0.5us at the
tail drain.

Note on timing variance: the device clock is a per-process lottery (~227ns vs
~272ns per matmul observed for identical NEFFs, all engines scaling together);
comparisons between kernel versions must be made on matmul-duration-normalized
traces or best-of-N runs.
"""

import math

import numpy as np

import concourse.mybir as mybir
import concourse.tile as tile
from concourse import bacc, bass_isa
from concourse.bass_utils import run_bass_kernel_spmd

P = 128          # partitions
S = 2048         # sequence length
DM = 512         # d_model == d_attn == d_value
ND = DM // P     # 4  d-model chunks
NS = S // P      # 16 sequence blocks
QC = 512         # q-chunk width for fused score/PV stages
NQC = S // QC    # 4
NEC = DM // P    # 4  e-chunks of the output
N_WARM_TINY = 16  # [1,1] warmup matmuls issued while the zero warm tiles memset
N_WARM_BIG = 12   # full-array [128x128x512] warmup matmuls on zeros: exercises
                  # the whole PE (power-level DVFS ramp signal, not just
                  # busy-time) while the first 1.25MB of inputs land in SBUF

F32 = mybir.dt.float32
F32R = mybir.dt.float32r
BF16 = mybir.dt.bfloat16

# 'f32r' (default): f32 storage, float32r matmuls.  'bf16': bf16 storage+matmuls.
MODE = "f32r"

_NC_CACHE = {}


def _build(mode):
    # tensors feeding the tensor engine carry the matmul dtype: the BIR
    # verifier requires fp32r matmul operands to be *produced* as float32r
    sb_dt = BF16 if mode == "bf16" else F32R
    nc = bacc.Bacc()

    # xt and outT live in DRAM pre-tiled chunk-major (host does the reshapes):
    # row ((i*NQC + sc)*P + p) of xt_d holds xt[i*P + p, sc*QC : (sc+1)*QC],
    # so every [P, QC] chunk DMA is one dense 256KB block instead of 128
    # 2KB lines strided 8KB apart — measurably better HBM burst efficiency
    # in the DMA-gated early window
    xt_d = nc.dram_tensor("xt", [ND * NQC * P, QC], sb_dt, kind="ExternalInput")
    # wm stays row-major [D, D]: splitting it into per-(i,j) dense blocks was
    # measured NET-WORSE (-3.5us) — the 8 extra dma_starts add per-queue
    # re-arm overhead that delays xt sc1-3/wv behind them
    wm_d = nc.dram_tensor("wm", [DM, DM], sb_dt, kind="ExternalInput")
    wv_d = nc.dram_tensor("wv", [DM, DM], sb_dt, kind="ExternalInput")
    outT_d = nc.dram_tensor("outT", [NEC * NQC * P, QC], F32, kind="ExternalOutput")

    mm = nc.tensor.matmul

    # f32r outputs on DVE ops trip the low-precision guard; actual matmul
    # accumulation stays in fp32 PSUM throughout.
    with nc.allow_low_precision(reason="fp32r operand rounding; PSUM accumulation is fp32"), \
         tile.TileContext(nc) as tc:
        with tc.tile_pool(name="consts", bufs=1) as consts:
            # persistent SBUF tensors (distinct tags so nothing shares slots)
            wm_sb = [consts.tile([P, DM], sb_dt, name=f"wm{i}", tag=f"wm{i}") for i in range(ND)]
            wv_sb = [consts.tile([P, DM], sb_dt, name=f"wv{i}", tag=f"wv{i}") for i in range(ND)]
            xt_sb = [consts.tile([P, S], sb_dt, name=f"xt{i}", tag=f"xt{i}") for i in range(ND)]
            qt_sb = [consts.tile([P, S], sb_dt, name=f"qt{j}", tag=f"qt{j}") for j in range(ND)]
            v_sb = [consts.tile([P, DM], sb_dt, name=f"v{b}", tag=f"v{b}") for b in range(NS)]
            # fp32 ones used by the tiny warmup matmuls (fp32r forbids
            # free-dim-1 matmuls)
            warm_src = consts.tile([P, 1], F32, name="warm_src", tag="warm_src")
            nc.vector.memset(warm_src, 1.0)
            # zero [P,P]x[P,QC] operands for the full-array warmup matmuls
            # (memset can't write f32r, so memset f32 then cast)
            warm_lhs_f = consts.tile([P, P], F32, name="warm_lhs_f", tag="warm_lhs_f")
            warm_rhs_f = consts.tile([P, QC], F32, name="warm_rhs_f", tag="warm_rhs_f")
            warm_lhs = consts.tile([P, P], sb_dt, name="warm_lhs", tag="warm_lhs")
            warm_rhs = consts.tile([P, QC], sb_dt, name="warm_rhs", tag="warm_rhs")
            nc.vector.memset(warm_lhs_f, 0.0)
            nc.vector.memset(warm_rhs_f, 0.0)
            nc.vector.tensor_copy(warm_lhs, warm_lhs_f)
            nc.vector.tensor_copy(warm_rhs, warm_rhs_f)
            # preload the ACT Exp table during stage A — otherwise the first
            # exp of the scores stage pays the ~1.3us table load inline
            exp_warm = consts.tile([P, 1], F32, name="exp_warm", tag="exp_warm")
            nc.scalar.activation(out=exp_warm, in_=warm_src,
                                 func=mybir.ActivationFunctionType.Exp)

            # input DMAs in first-use order: the first gT psum group needs
            # only wm's j0 columns + the first xt chunk, so those go first
            def xt_rows(i, sc):
                return slice((i * NQC + sc) * P, (i * NQC + sc + 1) * P)

            # xt-sc0 ahead of wm-rest: xt's first chunk gates the FIRST real
            # matmuls right at warmup end (loading all of wm first was
            # measured to starve them 2.9us AND let the clock decay); the
            # later-needed wm columns ride behind and their residual wait is
            # bridged by pad warmups below
            for i in range(ND):
                nc.sync.dma_start(out=wm_sb[i][:, 0:P], in_=wm_d[i * P:(i + 1) * P, 0:P])
            for i in range(ND):
                nc.sync.dma_start(out=xt_sb[i][:, 0:QC], in_=xt_d[xt_rows(i, 0), :])
            for i in range(ND):
                nc.sync.dma_start(out=wm_sb[i][:, P:DM], in_=wm_d[i * P:(i + 1) * P, P:DM])
            for sc in range(1, NQC):
                for i in range(ND):
                    nc.sync.dma_start(
                        out=xt_sb[i][:, sc * QC:(sc + 1) * QC],
                        in_=xt_d[xt_rows(i, sc), :],
                    )
            for i in range(ND):
                nc.sync.dma_start(out=wv_sb[i], in_=wv_d[i * P:(i + 1) * P, :])

            # ---- stage A: g projection (s-chunk-major: the first groups
            # only need wm's j0 columns + the first xt chunk) ----------------
            # psA takes 4 banks and is released before psO opens; psS is
            # opened while psA is still live so it gets never-used banks and
            # carries NO dependency on psA's release (a pool release waits on
            # ALL of the pool's accessors, which otherwise stalls the first
            # scores matmul behind the last stage-A copy)
            from contextlib import ExitStack as _ExitStack
            with (
                tc.tile_pool(name="psS", bufs=3, space="PSUM") as psS,
            ):
                _psa_stack = _ExitStack()
                psA = _psa_stack.enter_context(tc.tile_pool(name="psA", bufs=4, space="PSUM"))
                # PE warmup: tiny matmuls bridge the ~1us until the zero warm
                # tiles are memset, then full-array matmuls keep the whole PE
                # drawing power while inputs stream in, so the HAM clock is
                # ramped when real matmuls start.  (shares the psA tag/slots)
                warm = psA.tile([1, 1], F32, name="warm", tag="psA")
                for w in range(N_WARM_TINY):
                    mm(warm, warm_src, warm_src, start=True, stop=True)
                for w in range(N_WARM_BIG):
                    ps_w = psA.tile([P, QC], F32, name="ps_w", tag="psA")
                    mm(ps_w, warm_lhs, warm_rhs, start=True, stop=True)

                def g_group(sc, j):
                    # gT[:, sc-chunk] j-block; copies alternate ACT/DVE so
                    # neither engine queues deep behind the scores-stage exps
                    ps = psA.tile([P, QC], F32, name="psA", tag="psA")
                    for i in range(ND):
                        mm(ps, wm_sb[i][:, j * P:(j + 1) * P],
                           xt_sb[i][:, sc * QC:(sc + 1) * QC],
                           start=(i == 0), stop=(i == ND - 1))
                    copy_op = nc.scalar.copy if j % 2 == 0 else nc.vector.tensor_copy
                    copy_op(qt_sb[j][:, sc * QC:(sc + 1) * QC], ps)

                # only gT's first s-chunk is computed up front: scores(qc=0)
                # needs just that, and the remaining g groups are threaded
                # into the qc=0 scores loop below so the PE consumes xt at
                # the rate the input DMAs deliver it instead of stalling.
                # Two pad warmups after the first group bridge the systematic
                # ~1.3us wait for wm's later columns (a DMA delivery lull) —
                # an idle PE there was measured to drop the clock back to
                # mid-pstate
                for j in range(ND):
                    g_group(0, j)
                    if j == 0:
                        # 2 pads measured best; 5 just moved the residual
                        # DMA-floor wait elsewhere
                        for w in range(2):
                            ps_w = psA.tile([P, QC], F32, name="ps_w", tag="psA")
                            mm(ps_w, warm_lhs, warm_rhs, start=True, stop=True)

                # ---- stages B+C: scores -> exp -> denominators -> PV ------
                _pso_stack = _ExitStack()
                with (
                    tc.tile_pool(name="ptp", bufs=1) as ptp,
                    tc.tile_pool(name="work", bufs=2) as work,
                    tc.tile_pool(name="outp", bufs=3) as outp,
                ):
                    psO = None
                    for qc in range(NQC):
                        qs = slice(qc * QC, (qc + 1) * QC)
                        pt = ptp.tile([P, NS, QC], sb_dt, name="pt", tag="pt")
                        # partial k-sums of p, computed on the DVE in four
                        # quarters so the last one lands just after the scores
                        # finish and the combined sum is ready for the GpSimd
                        # partition_all_reduce before the PV norms need 1/l
                        h1 = work.tile([P, QC, 1], F32, name="h1", tag="h1", bufs=1)
                        h2 = work.tile([P, QC, 1], F32, name="h2", tag="h2", bufs=1)
                        NQ4 = NS // 4
                        for kb in range(NS):
                            ps_s = psS.tile([P, QC], F32, name="ps_s", tag="ps_s")
                            for j in range(ND):
                                mm(ps_s, xt_sb[j][:, kb * P:(kb + 1) * P], qt_sb[j][:, qs],
                                   start=(j == 0), stop=(j == ND - 1))
                            nc.scalar.activation(out=pt[:, kb, :], in_=ps_s,
                                                 func=mybir.ActivationFunctionType.Exp)
                            if kb == NQ4 - 1:
                                nc.vector.reduce_sum(
                                    out=h1, in_=pt[:, 0:NQ4, :].rearrange("p b q -> p q b"),
                                    axis=mybir.AxisListType.X)
                            elif kb == 2 * NQ4 - 1:
                                nc.vector.reduce_sum(
                                    out=h2, in_=pt[:, NQ4:2 * NQ4, :].rearrange("p b q -> p q b"),
                                    axis=mybir.AxisListType.X)
                                nc.vector.tensor_add(h1[:, :, 0], h1[:, :, 0], h2[:, :, 0])
                            elif kb == 3 * NQ4 - 1:
                                nc.vector.reduce_sum(
                                    out=h2, in_=pt[:, 2 * NQ4:3 * NQ4, :].rearrange("p b q -> p q b"),
                                    axis=mybir.AxisListType.X)
                                nc.vector.tensor_add(h1[:, :, 0], h1[:, :, 0], h2[:, :, 0])
                            if qc == 0 and kb % NQ4 == NQ4 - 1 and kb != NS - 1:
                                # thread the next gT s-chunk between score
                                # blocks: fills the PE while xt streams in
                                for j in range(ND):
                                    g_group(kb // NQ4 + 1, j)
                                if kb == NS - NQ4 - 1:
                                    # all g groups emitted: free psA's four
                                    # banks and only now open psO on them
                                    _psa_stack.close()
                                    psO = _pso_stack.enter_context(
                                        tc.tile_pool(name="psO", bufs=5, space="PSUM"))

                        if qc == 0:
                            # v-projection, emitted here so it fills the PE while
                            # the qc=0 exps finish (PV(0) depends on all of them).
                            # Emission is rotated so v_sb[0..5] — the tiles the
                            # first PV group consumes during its first ~1.4us —
                            # are produced mid-stream, not last: their copies are
                            # done before PV(0) streams past them, so PV starts
                            # the instant the last v matmul retires
                            for b in [(i + 6) % NS for i in range(NS)]:
                                psv = psO.tile([P, DM], F32, name="psv", tag="ps_o")
                                for i in range(ND):
                                    mm(psv, xt_sb[i][:, b * P:(b + 1) * P], wv_sb[i],
                                       start=(i == 0), stop=(i == ND - 1))
                                # ALL copies on ACT (idle in this window — the
                                # qc=0 exps are long done): putting half on the
                                # DVE was measured to stall PV(0)'s first PSUM
                                # bank ~550ns AND push the Q4-reduce/reciprocal
                                # chain to the wire, because the DVE drains its
                                # queue in order behind the trailing v copies
                                nc.scalar.copy(v_sb[b], psv)

                        nc.vector.reduce_sum(
                            out=h2, in_=pt[:, 3 * NQ4:NS, :].rearrange("p b q -> p q b"),
                            axis=mybir.AxisListType.X)
                        nc.vector.tensor_add(h1[:, :, 0], h1[:, :, 0], h2[:, :, 0])

                        # denominator: l[q] = sum over partitions of h1, done as
                        # a GpSimd partition_all_reduce (idle engine, SBUF-only)
                        # into a broadcast layout, then one fast DVE reciprocal.
                        # No PE work and no 3.3us exact-reciprocal serialization;
                        # ~51-ULP accuracy is far inside the error budget.
                        l_bc = work.tile([P, QC], F32, name="l_bc", tag="l_bc", bufs=1)
                        nc.gpsimd.partition_all_reduce(
                            l_bc, h1[:, :, 0], channels=P,
                            reduce_op=bass_isa.ReduceOp.add)
                        r_bc = work.tile([P, QC], F32, name="r_bc", tag="r_bc")
                        nc.vector.reciprocal_approx_fast(out=r_bc, in_=l_bc)
                        last = qc == NQC - 1
                        # PV: outT[e, q] = sum_k v[k, e] * p[k, q]; each chunk is
                        # normalized on the DVE and DMA'd out as soon as its PSUM
                        # group completes
                        for ec in range(NEC):
                            if last and ec == NEC - 1:
                                # final output tile in two column halves (separate
                                # PSUM banks — a shared bank would serialize on the
                                # first half's norm read): the first half's
                                # norm+DMA overlap the second half's matmuls
                                for h in range(2):
                                    hs = slice(h * (QC // 2), (h + 1) * (QC // 2))
                                    ps_h = psO.tile([P, QC // 2], F32, name="ps_h", tag="ps_o")
                                    for kb in range(NS):
                                        mm(ps_h, v_sb[kb][:, ec * P:(ec + 1) * P],
                                           pt[:, kb, hs], start=(kb == 0), stop=(kb == NS - 1))
                                    out_h = outp.tile([P, QC // 2], F32, name="out_h", tag="out_h")
                                    nc.vector.tensor_mul(out_h, ps_h, r_bc[:, hs])
                                    nc.sync.dma_start(
                                        out=outT_d[(ec * NQC + qc) * P:(ec * NQC + qc + 1) * P, hs],
                                        in_=out_h)
                            else:
                                ps_o = psO.tile([P, QC], F32, name="ps_o", tag="ps_o")
                                for kb in range(NS):
                                    mm(ps_o, v_sb[kb][:, ec * P:(ec + 1) * P], pt[:, kb, :],
                                       start=(kb == 0), stop=(kb == NS - 1))
                                out_sb = outp.tile([P, QC], F32, name="out_sb", tag="out_sb")
                                nc.vector.tensor_mul(out_sb, ps_o, r_bc)
                                nc.sync.dma_start(
                                    out=outT_d[(ec * NQC + qc) * P:(ec * NQC + qc + 1) * P, :],
                                    in_=out_sb)
                    _pso_stack.close()

    nc.compile()
    return nc


def _get_nc(mode):
    if mode not in _NC_CACHE:
        _NC_CACHE[mode] = _build(mode)
    return _NC_CACHE[mode]


def _prep_in_maps(x, Wq, Wk, Wv, mode):
    if mode == "bf16":
        import ml_dtypes

        def cast(a):
            return np.ascontiguousarray(a).astype(ml_dtypes.bfloat16)
    else:
        def cast(a):
            return np.ascontiguousarray(a, dtype=np.float32)

    scale = 1.0 / math.sqrt(DM)
    # fold the whole score bilinear form into one matrix: scores = x (Wq^T Wk s) x^T
    wm_h = cast((np.asarray(Wq, np.float32).T @ np.asarray(Wk, np.float32)) * scale)
    wv_h = cast(np.asarray(Wv, np.float32).T)
    x = np.asarray(x, np.float32)

    def tile_xt(xb):
        # [D, S] -> chunk-major [(i, sc, p), q]: each [P, QC] chunk one dense block
        return cast(xb.T.reshape(ND, P, NQC, QC).transpose(0, 2, 1, 3)
                    .reshape(ND * NQC * P, QC))

    return [
        {"xt": tile_xt(x[b]), "wm": wm_h, "wv": wv_h}
        for b in range(x.shape[0])
    ]


def _untile_out(outT_tiled):
    # chunk-major [(ec, qc, p), q] -> [S, D] (one batch element, transposed back)
    return (outT_tiled.reshape(NEC, NQC, P, QC).transpose(0, 2, 1, 3)
            .reshape(DM, S).T)


def _run(in_maps, mode=None, **kw):
    mode = mode or MODE
    nc = _get_nc(mode)
    return run_bass_kernel_spmd(nc, in_maps, core_ids=list(range(len(in_maps))), **kw)


def kernel(x, Wq, Wk, Wv):
    in_maps = _prep_in_maps(x, Wq, Wk, Wv, MODE)
    res = _run(in_maps)
    out = np.stack([_untile_out(r["outT"]) for r in res.results])
    return np.ascontiguousarray(out, dtype=np.float32)
